# revision 25
# baseline (speedup 1.0000x reference)
"""Causal multi-head attention on 8 TRN2 NeuronCores.

Sharding: core c -> (batch b = c // 2, head-half hh = c % 2).
Each core computes QKV for its 8 heads over the full sequence of its batch,
causal flash attention, and a partial out-projection using its 512 rows of
w_out. The host sums the two partials per batch (the "all-reduce" of the
tensor-parallel out projection).

Layouts (per core, all fp32r for TensorE fast path):
  KT[j]  [128, 2048]  K^T for head pair j (head 2j rows 0:64, 2j+1 rows 64:128)
  V[t]   [128, 520]   V token-tile t, 8 heads x (64 cols + ones col) for the
                      softmax denominator ("ones trick")
  QTz_e/o[j] [128,512] per-q-chunk Q^T, zero-padded to K=128 so S matmuls
                      contract over 128 partitions (K=64 runs at half rate)

Shapes (hardcoded): B=4, T=2048, D=1024, H=16, HD=64.
"""
import sys

for _p in ('/opt/trn_rl_repo', '/root/.axon_site/_ro/trn_rl_repo'):
    if _p not in sys.path:
        sys.path.insert(0, _p)

import numpy as np

B, T, D = 4, 2048, 1024
H, HD = 16, 64
HPC = H // 2          # heads per core = 8
DPC = HPC * HD        # out-dims per core = 512
N_CORES = 8

_nc_cache = {}


def _build_nc():
    import concourse.bacc as bacc
    import concourse.mybir as mybir
    from concourse.tile import TileContext

    F32 = mybir.dt.float32
    F32R = mybir.dt.float32r
    AF = mybir.ActivationFunctionType
    ALU = mybir.AluOpType

    CH = 512              # phase-1 token chunk
    QC = 512              # phase-2 query chunk
    NKB = T // 128        # 16 k-blocks
    NQC = T // QC         # 4 query chunks
    NCH = T // CH         # 4 phase-1 chunks
    NDT = D // 128        # 8 input-dim tiles
    VW = HPC * (HD + 1)   # V tile width = 520

    nc = bacc.Bacc('TRN2', target_bir_lowering=False, debug=False)
    xT_d = nc.dram_tensor('xT', [D, T], F32R, kind='ExternalInput')
    wq_d = nc.dram_tensor('wq', [D, DPC], F32R, kind='ExternalInput')
    wk_d = nc.dram_tensor('wk', [D, DPC], F32R, kind='ExternalInput')
    wv_d = nc.dram_tensor('wv', [D, DPC], F32R, kind='ExternalInput')
    wo_d = nc.dram_tensor('wo', [DPC, D], F32R, kind='ExternalInput')
    ones8_d = nc.dram_tensor('ones8', [128, HPC], F32R, kind='ExternalInput')
    zer_d = nc.dram_tensor('zer', [64, 512], F32R, kind='ExternalInput')
    po_d = nc.dram_tensor('po', [T, D], F32, kind='ExternalOutput')

    with nc.allow_low_precision(reason='fp32r matmuls by design'), \
            TileContext(nc) as tc:
        with (
            tc.tile_pool(name='kt', bufs=1) as kt_pool,
            tc.tile_pool(name='vv', bufs=1) as v_pool,
            tc.tile_pool(name='small', bufs=2) as sm_pool,
            tc.tile_pool(name='wq', bufs=1) as wq_pool,
            tc.tile_pool(name='qtz', bufs=2) as qtz_pool,
        ):
            WQ = [wq_pool.tile([128, DPC], F32R, tag=f'wq{d}',
                               name=f'wqs{d}') for d in range(NDT)]

            def alloc_qtz(c):
                QTe = [qtz_pool.tile([128, QC], F32R, tag=f'qte{j}',
                                     name=f'qte{j}') for j in range(4)]
                QTo = [qtz_pool.tile([128, QC], F32R, tag=f'qto{j}',
                                     name=f'qto{j}') for j in range(4)]
                if c < 2:   # init the zero rows once per pool slot
                    for j in range(4):
                        nc.sync.dma_start(QTe[j][64:128, :], zer_d[:, :])
                        nc.sync.dma_start(QTo[j][0:64, :], zer_d[:, :])
                return QTe, QTo

            # pre-warm the ACT exp table during phase 1 so the first real
            # exp doesn't pay the ~2.7us table load
            warm = sm_pool.tile([1, 16], F32, tag='warm', bufs=1)
            warm2 = sm_pool.tile([2, 16], F32, tag='warm2', bufs=1)
            nc.vector.memset(warm[:, :], 0.0)
            nc.scalar.activation(warm[:, :], warm[:, :], AF.Exp)
            nc.gpsimd.affine_select(
                out=warm[:, :], in_=warm[:, :], compare_op=ALU.is_ge,
                fill=0.0, base=0, channel_multiplier=-1, pattern=[[1, 16]])
            nc.gpsimd.partition_broadcast(warm2[:, :], warm[:, :])

            KT = [kt_pool.tile([128, T], F32R, tag=f'kt{j}', name=f'kt{j}')
                  for j in range(4)]
            V = [v_pool.tile([128, VW], F32R, tag=f'v{t}', name=f'v{t}')
                 for t in range(NKB)]

            # ---------------- Phase 1: K and V projections ----------------
            with (
                tc.tile_pool(name='wkv', bufs=1) as w_pool,
                tc.tile_pool(name='xs', bufs=2) as x_pool,
                tc.tile_pool(name='ps1', bufs=3, space='PSUM') as ps1,
            ):
                WK = [w_pool.tile([128, DPC], F32R, tag=f'wk{d}',
                                  name=f'wks{d}') for d in range(NDT)]
                WV = [w_pool.tile([128, DPC], F32R, tag=f'wv{d}',
                                  name=f'wvs{d}') for d in range(NDT)]
                xs0 = [x_pool.tile([128, CH], F32R, tag=f'x{d}',
                                   name=f'xs{d}') for d in range(NDT)]
                xs1 = [x_pool.tile([128, CH], F32R, tag=f'x{d}',
                                   name=f'xs{d}b') for d in range(NDT)]
                for d in range(NDT):
                    nc.sync.dma_start(xs0[d][:, :], xT_d[d*128:(d+1)*128, 0:CH])
                    nc.sync.dma_start(WK[d][:, :], wk_d[d*128:(d+1)*128, :])
                for d in range(NDT):
                    nc.sync.dma_start(WV[d][:, :], wv_d[d*128:(d+1)*128, :])
                for d in range(NDT):
                    nc.sync.dma_start(xs1[d][:, :],
                                      xT_d[d*128:(d+1)*128, CH:2*CH])
                for d in range(NDT):
                    nc.sync.dma_start(WQ[d][:, :], wq_d[d*128:(d+1)*128, :])

                for t in range(NKB):
                    vt3 = V[t].rearrange('p (h c) -> p h c', c=HD + 1)
                    nc.sync.dma_start(vt3[:, :, HD], ones8_d[:, :])
                qtz_cache = {}
                for c in range(NCH):
                    if c == 0:
                        xs = xs0
                    elif c == 1:
                        xs = xs1
                    else:
                        xs = [x_pool.tile([128, CH], F32R, tag=f'x{d}',
                                          name=f'xs{d}') for d in range(NDT)]
                        for d in range(NDT):
                            nc.sync.dma_start(
                                xs[d][:, :],
                                xT_d[d*128:(d+1)*128, c*CH:(c+1)*CH])
                    # KT: out [128 dout, CH tok]
                    for j in range(4):
                        pp = ps1.tile([128, CH], F32, tag='p1')
                        for d in range(NDT):
                            nc.tensor.matmul(
                                pp[:, :],
                                lhsT=WK[d][:, j*128:(j+1)*128],
                                rhs=xs[d][:, :],
                                start=(d == 0), stop=(d == NDT - 1))
                        nc.vector.tensor_copy(
                            KT[j][:, c*CH:(c+1)*CH], pp[:, :])
                    # V: out [128 tok, DPC dout]
                    for tt in range(CH // 128):
                        pv = ps1.tile([128, DPC], F32, tag='pv')
                        for d in range(NDT):
                            nc.tensor.matmul(
                                pv[:, :],
                                lhsT=xs[d][:, tt*128:(tt+1)*128],
                                rhs=WV[d][:, :],
                                start=(d == 0), stop=(d == NDT - 1))
                        vt3 = V[c*(CH // 128) + tt].rearrange(
                            'p (h c) -> p h c', c=HD + 1)
                        nc.vector.tensor_copy(
                            vt3[:, :, 0:HD],
                            pv.rearrange('p (h c) -> p h c', c=HD))
                    # Q^T (zero-padded) for the first two attention chunks,
                    # reusing the streamed x chunk (CH == QC)
                    if c < 2:
                        QTe, QTo = alloc_qtz(c)
                        qtz_cache[c] = (QTe, QTo)
                        for j in range(4):
                            pq = ps1.tile([128, QC], F32, tag='p1', name='pq')
                            for d in range(NDT):
                                nc.tensor.matmul(
                                    pq[:, :],
                                    lhsT=WQ[d][:, j*128:(j+1)*128],
                                    rhs=xs[d][:, :],
                                    start=(d == 0), stop=(d == NDT - 1))
                            nc.vector.tensor_copy(
                                QTe[j][0:64, :], pq[0:64, :])
                            nc.vector.tensor_copy(
                                QTo[j][64:128, :], pq[64:128, :])

            # ------------- Phase 2 + 3: attention + out-proj -------------
            with (
                tc.tile_pool(name='wo', bufs=1) as wo_pool,
                tc.tile_pool(name='xq', bufs=1) as xq_pool,
                tc.tile_pool(name='ao', bufs=2) as ao_pool,
                tc.tile_pool(name='pt', bufs=3) as pt_pool,
                tc.tile_pool(name='osb', bufs=2) as osb_pool,
                tc.tile_pool(name='ps_s', bufs=2, space='PSUM') as ps_s,
                tc.tile_pool(name='ps_ot', bufs=4, space='PSUM') as ps_ot,
            ):
                WO = [wo_pool.tile([128, D], F32R, tag=f'wo{d}',
                                   name=f'wos{d}') for d in range(4)]
                for d in range(4):
                    nc.sync.dma_start(WO[d][:, :], wo_d[d*128:(d+1)*128, :])

                for c in range(NQC):
                    q0 = c * QC
                    nkb = (q0 + QC) // 128      # causal k-blocks this chunk
                    QTe, QTo = qtz_cache.pop(c)
                    if c + 1 < NQC and c >= 1:
                        # compute Q^T for chunk c+1 (overlaps this chunk's
                        # attention; copies on DVE to stay off ACT's FIFO)
                        nq0 = (c + 1) * QC
                        xq = [xq_pool.tile([128, QC], F32R, tag=f'xq{d}',
                                           name=f'xq{d}') for d in range(NDT)]
                        for d in range(NDT):
                            nc.sync.dma_start(
                                xq[d][:, :],
                                xT_d[d*128:(d+1)*128, nq0:nq0+QC])
                        nQTe, nQTo = alloc_qtz(c + 1)
                        qtz_cache[c + 1] = (nQTe, nQTo)
                        for j in range(4):
                            pq = ps_ot.tile([128, QC], F32, tag='ot',
                                            name='pq')
                            for d in range(NDT):
                                nc.tensor.matmul(
                                    pq[:, :],
                                    lhsT=WQ[d][:, j*128:(j+1)*128],
                                    rhs=xq[d][:, :],
                                    start=(d == 0), stop=(d == NDT - 1))
                            nc.vector.tensor_copy(nQTe[j][0:64, :],
                                                  pq[0:64, :])
                            nc.vector.tensor_copy(nQTo[j][64:128, :],
                                                  pq[64:128, :])

                    ao = [ao_pool.tile([128, QC], F32R, tag=f'ao{j}',
                                       name=f'ao{j}') for j in range(4)]
                    for j in range(4):            # head pair (2j, 2j+1)
                        h0, h1 = 2*j, 2*j + 1
                        ot0 = ps_ot.tile([HD + 1, QC], F32, tag='ot',
                                         name='ot0')
                        ot1 = ps_ot.tile([HD + 1, QC], F32, tag='ot',
                                         name='ot1')
                        pend = None
                        for kbp in range(nkb // 2):
                            ka, kB = 2*kbp, 2*kbp + 1
                            lo_a = max(0, ka*128 - q0)
                            le_a = min(lo_a, QC - 256)
                            lo_b = max(0, kB*128 - q0)
                            le_b = min(lo_b, QC - 256)
                            s0 = ps_s.tile([128, 2*QC], F32, tag='s',
                                           name='s0')
                            s1 = ps_s.tile([128, 2*QC], F32, tag='s',
                                           name='s1')
                            pt0 = pt_pool.tile([128, 2*QC], F32R, tag='pt',
                                               name='pt0')
                            pt1 = pt_pool.tile([128, 2*QC], F32R, tag='pt',
                                               name='pt1')
                            ksa = KT[j][:, ka*128:(ka+1)*128]
                            ksb = KT[j][:, kB*128:(kB+1)*128]
                            nc.tensor.matmul(
                                s0[:, le_a:QC], lhsT=ksa,
                                rhs=QTe[j][:, le_a:QC],
                                start=True, stop=True)
                            nc.tensor.matmul(
                                s1[:, le_a:QC], lhsT=ksa,
                                rhs=QTo[j][:, le_a:QC],
                                start=True, stop=True)
                            nc.tensor.matmul(
                                s0[:, QC+le_b:2*QC], lhsT=ksb,
                                rhs=QTe[j][:, le_b:QC],
                                start=True, stop=True)
                            nc.tensor.matmul(
                                s1[:, QC+le_b:2*QC], lhsT=ksb,
                                rhs=QTo[j][:, le_b:QC],
                                start=True, stop=True)
                            if pend is not None:
                                for (pk, pl, pc0), ppt in pend:
                                    nc.tensor.matmul(
                                        ot0[:, pl:QC],
                                        lhsT=V[pk][:, (HD+1)*h0:
                                                   (HD+1)*(h0+1)],
                                        rhs=ppt[0][:, pc0+pl:pc0+QC],
                                        start=(pk == 0), stop=False)
                                    nc.tensor.matmul(
                                        ot1[:, pl:QC],
                                        lhsT=V[pk][:, (HD+1)*h1:
                                                   (HD+1)*(h1+1)],
                                        rhs=ppt[1][:, pc0+pl:pc0+QC],
                                        start=(pk == 0), stop=False)
                            nc.scalar.activation(
                                pt0[:, le_a:2*QC], s0[:, le_a:2*QC], AF.Exp)
                            nc.scalar.activation(
                                pt1[:, le_a:2*QC], s1[:, le_a:2*QC], AF.Exp)
                            for kx, lox, lex, c0 in ((ka, lo_a, le_a, 0),
                                                     (kB, lo_b, le_b, QC)):
                                if kx*128 >= q0:   # causal mask on diag
                                    for ptx in (pt0, pt1):
                                        nc.gpsimd.affine_select(
                                            out=ptx[:, c0+lex:c0+lox+128],
                                            in_=ptx[:, c0+lex:c0+lox+128],
                                            compare_op=ALU.is_ge, fill=0.0,
                                            base=lex - lox,
                                            channel_multiplier=-1,
                                            pattern=[[1, lox + 128 - lex]])
                            pend = [((ka, le_a, 0), (pt0, pt1)),
                                    ((kB, le_b, QC), (pt0, pt1))]
                        for (pk, pl, pc0), ppt in pend:
                            nc.tensor.matmul(
                                ot0[:, pl:QC],
                                lhsT=V[pk][:, (HD+1)*h0:(HD+1)*(h0+1)],
                                rhs=ppt[0][:, pc0+pl:pc0+QC],
                                start=(pk == 0),
                                stop=(pk == nkb - 1))
                            nc.tensor.matmul(
                                ot1[:, pl:QC],
                                lhsT=V[pk][:, (HD+1)*h1:(HD+1)*(h1+1)],
                                rhs=ppt[1][:, pc0+pl:pc0+QC],
                                start=(pk == 0),
                                stop=(pk == nkb - 1))
                        # normalize both heads of the pair
                        rp0 = sm_pool.tile([1, QC], F32, tag='rp0', bufs=2)
                        rp1 = sm_pool.tile([1, QC], F32, tag='rp1', bufs=2)
                        din0 = sm_pool.tile([1, QC], F32, tag='din0', bufs=2)
                        din1 = sm_pool.tile([1, QC], F32, tag='din1', bufs=2)
                        nc.vector.tensor_copy(din0[:, :], ot0[HD:HD+1, :])
                        nc.vector.tensor_copy(din1[:, :], ot1[HD:HD+1, :])
                        nc.vector.reciprocal_approx_fast(
                            out=rp0[:, :], in_=din0[:, :])
                        nc.vector.reciprocal_approx_fast(
                            out=rp1[:, :], in_=din1[:, :])
                        rbs0 = sm_pool.tile([HD, QC], F32, tag='rbs0', bufs=2)
                        rbs1 = sm_pool.tile([HD, QC], F32, tag='rbs1', bufs=2)
                        nc.gpsimd.partition_broadcast(rbs0[:, :], rp0[:, :])
                        nc.gpsimd.partition_broadcast(rbs1[:, :], rp1[:, :])
                        nc.vector.tensor_tensor(
                            out=ao[j][0:HD, :], in0=ot0[0:HD, :],
                            in1=rbs0[:, :], op=ALU.mult)
                        nc.vector.tensor_tensor(
                            out=ao[j][HD:128, :], in0=ot1[0:HD, :],
                            in1=rbs1[:, :], op=ALU.mult)
                    # fused partial out-projection for this q-chunk
                    for qt in range(QC // 128):
                        os = osb_pool.tile([128, D], F32, tag='os', name='os')
                        pj = ps_s.tile([128, 2*QC], F32, tag='s', name='pj')
                        for half in range(2):
                            for d in range(4):
                                nc.tensor.matmul(
                                    pj[:, half*512:(half+1)*512],
                                    lhsT=ao[d][:, qt*128:(qt+1)*128],
                                    rhs=WO[d][:, half*512:(half+1)*512],
                                    start=(d == 0), stop=(d == 3))
                        nc.vector.tensor_copy(os[:, :], pj[:, :])
                        nc.sync.dma_start(
                            po_d[q0+qt*128:q0+(qt+1)*128, :], os[:, :])

    nc.compile()
    return nc


def _get_nc():
    if 'nc' not in _nc_cache:
        _nc_cache['nc'] = _build_nc()
    return _nc_cache['nc']


def kernel(x, w_qkv, w_out, _profile=False):
    from concourse.bass_utils import run_bass_kernel_spmd

    x = np.asarray(x, dtype=np.float32)
    w_qkv = np.asarray(w_qkv, dtype=np.float32)
    w_out = np.asarray(w_out, dtype=np.float32)

    nc = _get_nc()

    scale = np.float32(1.0 / np.sqrt(HD))
    ones8 = np.ones((128, HPC), np.float32)
    zer = np.zeros((64, 512), np.float32)
    in_maps = []
    for c in range(N_CORES):
        b, hh = c // 2, c % 2
        s, e = hh * DPC, (hh + 1) * DPC
        in_maps.append({
            'xT': np.ascontiguousarray(x[b].T),
            'wq': np.ascontiguousarray(w_qkv[:, s:e] * scale),
            'wk': np.ascontiguousarray(w_qkv[:, D+s:D+e]),
            'wv': np.ascontiguousarray(w_qkv[:, 2*D+s:2*D+e]),
            'wo': np.ascontiguousarray(w_out[s:e, :]),
            'ones8': ones8,
            'zer': zer,
        })

    res = run_bass_kernel_spmd(nc, in_maps, core_ids=list(range(N_CORES)),
                               trace=_profile)
    out = np.empty((B, T, D), np.float32)
    for b in range(B):
        out[b] = res.results[2*b]['po'] + res.results[2*b+1]['po']
    if _profile:
        return out, res
    return out
